# revision 16
# baseline (speedup 1.0000x reference)
"""HGRN attention Trainium2 kernel (v6).

Sharding: B*L (4 batches x 4096 tokens) split into 8 chunks of T=2048 tokens,
one per NeuronCore: core c = 2*b + half handles tokens [half*T, (half+1)*T) of
batch b. The gated linear recurrence h_t = sigmoid(f_t)*h_{t-1} + swiglu-input
runs per (batch, channel); the cross-chunk carry (h at the half boundary) is
exchanged with a tiny pairwise AllReduce and applied as h_local + cumprod*carry
(cumprod underflows to 0 in fp32 past ~130 steps, so only the first 256
columns of each odd chunk need the fixup).

Structure: phase 1 keeps x SBUF-resident (bf16), runs the et loop over the
full T per weight tile (Wi/Wf/Wg loaded exactly once, host pre-tiled bf16),
and — because rms is a per-time-column scalar that commutes through the
o-projection — computes the full gate product osb' = g*gnw*silu(h) inline
(g straight from PSUM), spilling osb' in bf16 (last two et tiles stay in
SBUF). Phase 2 is pure o-projection matmul streaming; rms multiplies the PSUM
result on the way out (DVE). Chunk 0 (the only one needing the carry fixup)
runs last so the ~60us AllReduce hides under chunks 1-3.

Engine discipline (the p75-latency lessons): the Act engine runs ONLY Sigmoid
in phase 1 — silu(x) is x*sigmoid(x) on DVE — so its activation table never
reloads; every copy/downcast lives on DVE; rsqrt is the (cheap, table-based)
Abs_reciprocal_sqrt activation executed in phase 2 where the Act engine is
otherwise idle, so no 3.4us DVE reciprocal ever blocks the y-store chain; the
ones-matmul for each rms chunk is deferred one chunk so its drain never stalls
the PE; chunk-1 osb strips are SBUF-copied and the first Wo tiles preloaded
into non-aliased SBUF during phase 1 so the o-projection starts the moment the
last projection matmul retires; per-dt y stores; o-proj accumulation starts
and ends on SBUF-resident et tiles; pf PSUM triple-buffered; first weight/x
loads split across parallel DMA queues. End-to-end rel err ~5e-3 (gate 2e-2).
"""
import numpy as np
import ml_dtypes

import concourse.bacc as bacc
import concourse.tile as tile
import concourse.mybir as mybir
from concourse.bass_utils import run_bass_kernel_spmd

B, L, D = 4, 4096, 2048
T = 2048                 # tokens per core
NCORE = 8
ET = DT = D // 128       # 16 tiles of 128 channels
CH = 512                 # time chunk (one PSUM bank)
NC = T // CH             # 4
CLEN = 256               # cumprod fixup length (0 in fp32 beyond this)
ETS = ET - 2             # et tiles spilled to DRAM (last 2 stay in SBUF)
NWOP = 2                 # Wo tiles preloaded during phase 1
EPS = 1e-5

F32 = mybir.dt.float32
BF16 = mybir.dt.bfloat16
AF = mybir.ActivationFunctionType
OP = mybir.AluOpType

_CACHE = {}


def _build():
    nc = bacc.Bacc("TRN2", target_bir_lowering=False, debug=False,
                   enable_asserts=True, num_devices=NCORE)
    xt_d = nc.dram_tensor("xt", [D, T], BF16, kind="ExternalInput")
    # host-pre-tiled weights: row block et is the lhsT tile [128, DT*128]
    wi_d = nc.dram_tensor("wi", [ET * 128, DT * 128], BF16, kind="ExternalInput")
    wf_d = nc.dram_tensor("wf", [ET * 128, DT * 128], BF16, kind="ExternalInput")
    wg_d = nc.dram_tensor("wg", [ET * 128, DT * 128], BF16, kind="ExternalInput")
    wo_d = nc.dram_tensor("wo", [DT * 128, ET * 128], BF16, kind="ExternalInput")
    gnw_d = nc.dram_tensor("gnw", [128, ET], F32, kind="ExternalInput")
    mask_d = nc.dram_tensor("mask", [128, 1], F32, kind="ExternalInput")
    yt_d = nc.dram_tensor("yt", [D, T], BF16, kind="ExternalOutput")

    with tile.TileContext(nc) as tc:
        with tc.tile_pool(name="persist", bufs=1) as pp, \
             tc.tile_pool(name="dram", bufs=1, space="DRAM") as dr, \
             tc.tile_pool(name="hg", bufs=2) as hgp, \
             tc.tile_pool(name="wr", bufs=1) as wr, \
             tc.tile_pool(name="pre2", bufs=1) as pre2:
            carry = pp.tile([128, ET], F32, tag="carry")
            recv = pp.tile([128, ET], F32, tag="recv")
            cin = pp.tile([128, ET], F32, tag="cin")
            gnw = pp.tile([128, ET], F32, tag="gnw")
            maskt = pp.tile([128, 1], F32, tag="mask")
            call = pp.tile([128, ET * CLEN], BF16, tag="call")
            h0sb = pp.tile([128, ET * CLEN], BF16, tag="h0")
            g0sb = pp.tile([128, ET * CLEN], BF16, tag="g0")
            acc = pp.tile([128, T], F32, tag="acc")
            rms = pp.tile([128, T], F32, tag="rms")
            onesb = pp.tile([128, 128], BF16, tag="ones")

            osb_sp = dr.tile([D, T], BF16, tag="osp")
            hl_i = dr.tile([128, ET], F32, tag="hli")
            hl_o = dr.tile([128, ET], F32, tag="hlo")

            nc.vector.memset(carry[:], 0.0)
            nc.vector.memset(onesb[:], 1.0)
            nc.vector.memset(acc[:], 0.0)

            osb_live = {}   # et -> SBUF tile for the unspilled tail ets
            pre1 = {}       # et -> SBUF strip of chunk-1 osb (pre-copied)
            wos = [None] * DT
            accbs = [None] * NC
            # m(c) = mean(g^2) + eps, persisted to phase 2 for rsqrt
            ms = [wr.tile([128, CH], F32, tag=f"m{c}", name=f"m{c}")
                  for c in range(NC)]

            def emit_sum(c, mp):
                # cross-channel sum of g^2 for time chunk c -> ms[c]
                S = mp.tile([128, CH], F32, tag="S")
                nc.tensor.matmul(S[:], onesb[:], accbs[c][:],
                                 start=True, stop=True)
                nc.vector.tensor_scalar(ms[c][:], S[:], 1.0 / D, EPS,
                                        OP.mult, OP.add)

            # ---------------- phase 1: proj + scan + fused gating -----------
            with tc.tile_pool(name="xp", bufs=1) as xp, \
                 tc.tile_pool(name="wp", bufs=2) as wp, \
                 tc.tile_pool(name="wk", bufs=2) as wk, \
                 tc.tile_pool(name="pjf", bufs=3, space="PSUM") as pjf, \
                 tc.tile_pool(name="pji", bufs=2, space="PSUM") as pji, \
                 tc.tile_pool(name="prms", bufs=1, space="PSUM") as prms:
                x_sb = xp.tile([128, DT * T], BF16, tag="x")
                xv = x_sb[:].rearrange("p (dt t) -> p dt t", t=T)
                xs_d = xt_d.ap().rearrange("(dt p) t -> p dt t", p=128)
                for et in range(ET):
                    ws = []
                    for nm, wd in (("wf", wf_d), ("wi", wi_d), ("wg", wg_d)):
                        w = wp.tile([128, DT * 128], BF16, tag=nm)
                        if et == 0 and nm == "wf":
                            # fine-grained first loads across parallel queues
                            nc.sync.dma_start(
                                w[:, 0:DT * 64],
                                wd.ap()[0:128, 0:DT * 64])
                            for q in range(4):
                                nc.sync.dma_start(
                                    xv[:, 4 * q:4 * (q + 1), 0:CH],
                                    xs_d[:, 4 * q:4 * (q + 1), 0:CH])
                            nc.sync.dma_start(
                                w[:, DT * 64:DT * 128],
                                wd.ap()[0:128, DT * 64:DT * 128])
                        else:
                            nc.sync.dma_start(
                                w[:], wd.ap()[et * 128:(et + 1) * 128, :])
                        ws.append(w)
                    wfv, wiv, wgv = ws
                    if et == 0:
                        for c in range(1, NC):
                            nc.sync.dma_start(xv[:, :, c * CH:(c + 1) * CH],
                                              xs_d[:, :, c * CH:(c + 1) * CH])
                        nc.sync.dma_start(gnw[:], gnw_d.ap()[:])
                        nc.sync.dma_start(maskt[:], mask_d.ap()[:])
                    if et == 1:
                        for dt in range(NWOP):
                            wo = pre2.tile([128, ET * 128], BF16, tag=f"wo{dt}")
                            nc.sync.dma_start(
                                wo[:], wo_d.ap()[dt * 128:(dt + 1) * 128, :])
                            wos[dt] = wo
                    o_et = hgp.tile([128, T], BF16, tag="osb")
                    for c in range(NC):
                        cs = slice(c * CH, (c + 1) * CH)
                        pf = pjf.tile([128, CH], F32, tag="pf")
                        pi = pji.tile([128, CH], F32, tag="pi")
                        pg = pji.tile([128, CH], F32, tag="pg")
                        for ps, w in ((pf, wfv), (pi, wiv), (pg, wgv)):
                            for dt in range(DT):
                                nc.tensor.matmul(
                                    ps[:], w[:, dt * 128:(dt + 1) * 128],
                                    x_sb[:, dt * T + c * CH:dt * T + (c + 1) * CH],
                                    start=(dt == 0), stop=(dt == DT - 1))
                        gate = wk.tile([128, CH], F32, tag="gate")
                        nc.scalar.activation(gate[:], pf[:], AF.Sigmoid)
                        sgi = wk.tile([128, CH], F32, tag="sgi")
                        nc.scalar.activation(sgi[:], pi[:], AF.Sigmoid)
                        omg = wk.tile([128, CH], F32, tag="omg")
                        nc.vector.tensor_scalar(omg[:], gate[:], -1.0, 1.0,
                                                OP.mult, OP.add)
                        sq = wk.tile([128, CH], F32, tag="sq")
                        nc.vector.tensor_mul(sq[:], pi[:], sgi[:])  # silu(i)
                        iin = wk.tile([128, CH], F32, tag="iin")
                        nc.vector.tensor_mul(iin[:], sq[:], omg[:])
                        htmp = wk.tile([128, CH], F32, tag="htmp")
                        nc.vector.tensor_tensor_scan(
                            htmp[:], gate[:], iin[:], carry[:, et:et + 1],
                            OP.mult, OP.add)
                        nc.vector.tensor_copy(carry[:, et:et + 1],
                                              htmp[:, CH - 1:CH])
                        if c == 0:
                            nc.vector.tensor_tensor_scan(
                                call[:, et * CLEN:(et + 1) * CLEN],
                                gate[:, 0:CLEN], gate[:, 0:CLEN], 1.0,
                                OP.mult, OP.bypass)
                            nc.vector.tensor_copy(
                                h0sb[:, et * CLEN:(et + 1) * CLEN],
                                htmp[:, 0:CLEN])
                        sgh = wk.tile([128, CH], F32, tag="sgh")
                        nc.scalar.activation(sgh[:], htmp[:], AF.Sigmoid)
                        swh = wk.tile([128, CH], F32, tag="swh")
                        nc.vector.tensor_mul(swh[:], htmp[:], sgh[:])
                        nc.vector.scalar_tensor_tensor(
                            o_et[:, cs], pg[:], gnw[:, et:et + 1], swh[:],
                            OP.mult, OP.mult)
                        gsq = wk.tile([128, CH], F32, tag="gsq")
                        nc.vector.tensor_copy(gsq[:], pg[:])
                        if c == 0:
                            nc.vector.tensor_copy(
                                g0sb[:, et * CLEN:(et + 1) * CLEN],
                                gsq[:, 0:CLEN])
                        nc.vector.tensor_mul(sq[:], gsq[:], gsq[:])
                        nc.vector.tensor_add(acc[:, cs], acc[:, cs], sq[:])
                        if et == ET - 1:
                            accb = hgp.tile([128, CH], BF16, tag="accb")
                            nc.vector.tensor_copy(accb[:], acc[:, cs])
                            accbs[c] = accb
                            if c >= 1:   # deferred: never stalls the PE
                                emit_sum(c - 1, prms)
                    if et < ETS:
                        nc.sync.dma_start(osb_sp[et * 128:(et + 1) * 128, :],
                                          o_et[:])
                        p1 = pre2.tile([128, CH], BF16, tag=f"p1s{et}")
                        nc.vector.tensor_copy(p1[:], o_et[:, CH:2 * CH])
                        pre1[et] = p1
                    else:
                        osb_live[et] = o_et

            # ---------------- phase 1.5: carry exchange ---------------------
            nc.sync.dma_start(hl_i[:], carry[:])
            nc.gpsimd.collective_compute(
                "AllReduce", OP.add,
                replica_groups=[[0, 1], [2, 3], [4, 5], [6, 7]],
                ins=[hl_i.opt()], outs=[hl_o.opt()])

            # ---------------- phase 2: output projection --------------------
            # chunk 0 (needs the carry fixup) last, so the collective hides
            corder = (1, 2, 3, 0)
            ets_order = [ET - 2] + list(range(ETS)) + [ET - 1]
            strips = {}
            with tc.tile_pool(name="wop", bufs=1) as wop, \
                 tc.tile_pool(name="oscp", bufs=2) as oscp, \
                 tc.tile_pool(name="w2", bufs=3) as w2, \
                 tc.tile_pool(name="ycp", bufs=3) as ycp, \
                 tc.tile_pool(name="yp", bufs=4, space="PSUM") as yp:
                for dt in range(NWOP, DT):
                    wo = wop.tile([128, ET * 128], BF16, tag=f"wo{dt}")
                    nc.sync.dma_start(wo[:], wo_d.ap()[dt * 128:(dt + 1) * 128, :])
                    wos[dt] = wo

                def load_strips(c):
                    for et in range(ETS):
                        t = oscp.tile([128, CH], BF16, tag=f"os{et}")
                        nc.sync.dma_start(
                            t[:], osb_sp[et * 128:(et + 1) * 128,
                                         c * CH:(c + 1) * CH])
                        strips[(c, et)] = t

                load_strips(2)
                # rsqrt on the (idle) Act engine, first-needed chunk first
                for c in (1, 2, 0):
                    nc.scalar.activation(rms[:, c * CH:(c + 1) * CH],
                                         ms[c][:], AF.Abs_reciprocal_sqrt)
                for ci, c in enumerate(corder):
                    cs = slice(c * CH, (c + 1) * CH)
                    if c == 2:
                        load_strips(3)
                    elif c == 3:
                        load_strips(0)
                    if c == 0:
                        # collective result -> cin, then fix chunk-0 prefixes
                        nc.sync.dma_start(recv[:], hl_o[:])
                        nc.vector.tensor_sub(recv[:], recv[:], carry[:])
                        nc.vector.tensor_scalar(cin[:], recv[:],
                                                maskt[:, 0:1], None, OP.mult)
                        for et in range(ET):
                            es = slice(et * CLEN, (et + 1) * CLEN)
                            hf = w2.tile([128, CLEN], F32, tag="hf")
                            nc.vector.tensor_copy(hf[:], h0sb[:, es])
                            cf = w2.tile([128, CLEN], F32, tag="cf")
                            nc.vector.tensor_copy(cf[:], call[:, es])
                            nc.vector.scalar_tensor_tensor(
                                hf[:], cf[:], cin[:, et:et + 1], hf[:],
                                OP.mult, OP.add)
                            swf = w2.tile([128, CLEN], F32, tag="swf")
                            nc.scalar.activation(swf[:], hf[:], AF.Silu)
                            gf0 = w2.tile([128, CLEN], F32, tag="gf0")
                            nc.vector.tensor_copy(gf0[:], g0sb[:, es])
                            dst = (strips[(0, et)][:, 0:CLEN] if et < ETS
                                   else osb_live[et][:, 0:CLEN])
                            nc.vector.scalar_tensor_tensor(
                                dst, gf0[:], gnw[:, et:et + 1], swf[:],
                                OP.mult, OP.mult)
                    for dt in range(DT):
                        ypt = yp.tile([128, CH], F32, tag="ypt")
                        for k, et in enumerate(ets_order):
                            if et >= ETS:
                                src = osb_live[et][:, cs]
                            elif c == 1:
                                src = pre1[et][:]
                            else:
                                src = strips[(c, et)][:]
                            nc.tensor.matmul(
                                ypt[:], wos[dt][:, et * 128:(et + 1) * 128],
                                src, start=(k == 0), stop=(k == ET - 1))
                        ych = ycp.tile([128, CH], BF16, tag="ych")
                        nc.vector.tensor_mul(ych[:], ypt[:], rms[:, cs])
                        nc.sync.dma_start(
                            yt_d.ap()[dt * 128:(dt + 1) * 128, cs], ych[:])
                        if ci == 0 and dt == 2:
                            # last rms chunk, off the hot path
                            S3 = yp.tile([128, CH], F32, tag="S")
                            nc.tensor.matmul(S3[:], onesb[:], accbs[3][:],
                                             start=True, stop=True)
                            nc.vector.tensor_scalar(ms[3][:], S3[:], 1.0 / D,
                                                    EPS, OP.mult, OP.add)
                            nc.scalar.activation(rms[:, 3 * CH:4 * CH],
                                                 ms[3][:],
                                                 AF.Abs_reciprocal_sqrt)
    nc.compile()
    return nc


def _get_nc():
    if "nc" not in _CACHE:
        _CACHE["nc"] = _build()
    return _CACHE["nc"]


def _tile_lhs(WT):
    """[K*128, M*128] host mat -> pre-tiled [M_blk*128, K_blk*128] so row block
    m is the lhsT tile [128, K*128] with columns grouped by contraction block."""
    KB = WT.shape[0] // 128
    MB = WT.shape[1] // 128
    t = WT.reshape(KB, 128, MB, 128).transpose(2, 1, 0, 3)
    return np.ascontiguousarray(t.reshape(MB * 128, KB * 128)
                                .astype(ml_dtypes.bfloat16))


def kernel(hidden_states, Wi, Wf, Wg, g_norm_weight, Wo, **_unused):
    nc = _get_nc()
    bf = ml_dtypes.bfloat16
    wiT = _tile_lhs(np.ascontiguousarray(Wi.T))
    wfT = _tile_lhs(np.ascontiguousarray(Wf.T))
    wgT = _tile_lhs(np.ascontiguousarray(Wg.T))
    woT = _tile_lhs(np.ascontiguousarray(Wo.T))
    gnw = np.ascontiguousarray(
        np.asarray(g_norm_weight, np.float32).reshape(ET, 128).T)
    in_maps = []
    for c in range(NCORE):
        b, half = c // 2, c % 2
        xt = np.ascontiguousarray(
            hidden_states[b, half * T:(half + 1) * T, :].T.astype(bf))
        mask = np.full((128, 1), float(half), np.float32)
        in_maps.append({"xt": xt, "wi": wiT, "wf": wfT, "wg": wgT,
                        "wo": woT, "gnw": gnw, "mask": mask})
    _CACHE["in_maps"] = in_maps
    res = run_bass_kernel_spmd(nc, in_maps, list(range(NCORE))).results
    y = np.empty((B, L, D), np.float32)
    for c in range(NCORE):
        b, half = c // 2, c % 2
        y[b, half * T:(half + 1) * T, :] = res[c]["yt"].astype(np.float32).T
    return y


# revision 20
# speedup vs baseline: 1.0050x; 1.0050x over previous
"""HGRN attention Trainium2 kernel (v6).

Sharding: B*L (4 batches x 4096 tokens) split into 8 chunks of T=2048 tokens,
one per NeuronCore: core c = 2*b + half handles tokens [half*T, (half+1)*T) of
batch b. The gated linear recurrence h_t = sigmoid(f_t)*h_{t-1} + swiglu-input
runs per (batch, channel); the cross-chunk carry (h at the half boundary) is
exchanged with a tiny pairwise AllReduce and applied as h_local + cumprod*carry
(cumprod underflows to 0 in fp32 past ~130 steps, so only the first 256
columns of each odd chunk need the fixup).

Structure: phase 1 keeps x SBUF-resident (bf16), runs the et loop over the
full T per weight tile (Wi/Wf/Wg loaded exactly once, host pre-tiled bf16),
and — because rms is a per-time-column scalar that commutes through the
o-projection — computes the full gate product osb' = g*gnw*silu(h) inline
(g straight from PSUM), spilling osb' in bf16 (last two et tiles stay in
SBUF). Phase 2 is pure o-projection matmul streaming; rms multiplies the PSUM
result on the way out (DVE). Chunk 0 (the only one needing the carry fixup)
runs last so the ~60us AllReduce hides under chunks 1-3.

Engine discipline (the p75-latency lessons): the Act engine runs ONLY Sigmoid
in phase 1 — silu(x) is x*sigmoid(x) on DVE — so its activation table never
reloads; every copy/downcast lives on DVE; rsqrt is the (cheap, table-based)
Abs_reciprocal_sqrt activation executed in phase 2 where the Act engine is
otherwise idle, so no 3.4us DVE reciprocal ever blocks the y-store chain; the
ones-matmul for each rms chunk is deferred one chunk so its drain never stalls
the PE; chunk-1 osb strips are SBUF-copied and the first Wo tiles preloaded
into non-aliased SBUF during phase 1 so the o-projection starts the moment the
last projection matmul retires; per-dt y stores; o-proj accumulation starts
and ends on SBUF-resident et tiles; pf PSUM triple-buffered; first weight/x
loads split across parallel DMA queues. End-to-end rel err ~5e-3 (gate 2e-2).
"""
import numpy as np
import ml_dtypes

import concourse.bacc as bacc
import concourse.tile as tile
import concourse.mybir as mybir
from concourse.bass_utils import run_bass_kernel_spmd

B, L, D = 4, 4096, 2048
T = 2048                 # tokens per core
NCORE = 8
ET = DT = D // 128       # 16 tiles of 128 channels
CH = 512                 # time chunk (one PSUM bank)
NC = T // CH             # 4
CLEN = 256               # cumprod fixup length (0 in fp32 beyond this)
ETS = ET - 2             # et tiles spilled to DRAM (last 2 stay in SBUF)
NWOP = 2                 # Wo tiles preloaded during phase 1
EPS = 1e-5

F32 = mybir.dt.float32
BF16 = mybir.dt.bfloat16
AF = mybir.ActivationFunctionType
OP = mybir.AluOpType

_CACHE = {}


def _build():
    nc = bacc.Bacc("TRN2", target_bir_lowering=False, debug=False,
                   enable_asserts=True, num_devices=NCORE)
    xt_d = nc.dram_tensor("xt", [D, T], BF16, kind="ExternalInput")
    # host-pre-tiled weights: row block et is the lhsT tile [128, DT*128]
    wi_d = nc.dram_tensor("wi", [ET * 128, DT * 128], BF16, kind="ExternalInput")
    wf_d = nc.dram_tensor("wf", [ET * 128, DT * 128], BF16, kind="ExternalInput")
    wg_d = nc.dram_tensor("wg", [ET * 128, DT * 128], BF16, kind="ExternalInput")
    wo_d = nc.dram_tensor("wo", [DT * 128, ET * 128], BF16, kind="ExternalInput")
    gnw_d = nc.dram_tensor("gnw", [128, ET], F32, kind="ExternalInput")
    mask_d = nc.dram_tensor("mask", [128, 1], F32, kind="ExternalInput")
    yt_d = nc.dram_tensor("yt", [D, T], BF16, kind="ExternalOutput")

    with tile.TileContext(nc) as tc:
        with tc.tile_pool(name="persist", bufs=1) as pp, \
             tc.tile_pool(name="dram", bufs=1, space="DRAM") as dr, \
             tc.tile_pool(name="hg", bufs=2) as hgp, \
             tc.tile_pool(name="wr", bufs=1) as wr, \
             tc.tile_pool(name="pre2", bufs=1) as pre2:
            carry = pp.tile([128, ET], F32, tag="carry")
            recv = pp.tile([128, ET], F32, tag="recv")
            cin = pp.tile([128, ET], F32, tag="cin")
            gnw = pp.tile([128, ET], F32, tag="gnw")
            maskt = pp.tile([128, 1], F32, tag="mask")
            call = pp.tile([128, ET * CLEN], BF16, tag="call")
            h0sb = pp.tile([128, ET * CLEN], BF16, tag="h0")
            g0sb = pp.tile([128, ET * CLEN], BF16, tag="g0")
            acc = pp.tile([128, T], F32, tag="acc")
            rms = pp.tile([128, T], F32, tag="rms")
            onesb = pp.tile([128, 128], BF16, tag="ones")

            osb_sp = dr.tile([D, T], BF16, tag="osp")
            hl_i = dr.tile([128, ET], F32, tag="hli")
            hl_o = dr.tile([128, ET], F32, tag="hlo")

            nc.vector.memset(carry[:], 0.0)
            nc.vector.memset(onesb[:], 1.0)
            nc.vector.memset(acc[:], 0.0)

            osb_live = {}   # et -> SBUF tile for the unspilled tail ets
            pre1 = {}       # et -> SBUF strip of chunk-1 osb (pre-copied)
            wos = [None] * DT
            accbs = [None] * NC
            # m(c) = mean(g^2) + eps, persisted to phase 2 for rsqrt
            ms = [wr.tile([128, CH], F32, tag=f"m{c}", name=f"m{c}")
                  for c in range(NC)]

            def emit_sum(c, mp):
                # cross-channel sum of g^2 for time chunk c -> ms[c].
                # Act (not DVE) drains the PSUM: its queue is shallow, so the
                # bank frees promptly for phase-2 reuse.
                S = mp.tile([128, CH], F32, tag="S")
                nc.tensor.matmul(S[:], onesb[:], accbs[c][:],
                                 start=True, stop=True)
                nc.scalar.activation(ms[c][:], S[:], AF.Copy,
                                     bias=EPS, scale=1.0 / D)

            # ---------------- phase 1: proj + scan + fused gating -----------
            with tc.tile_pool(name="xp", bufs=1) as xp, \
                 tc.tile_pool(name="wp", bufs=2) as wp, \
                 tc.tile_pool(name="wk", bufs=2) as wk, \
                 tc.tile_pool(name="pjf", bufs=3, space="PSUM") as pjf, \
                 tc.tile_pool(name="pji", bufs=2, space="PSUM") as pji, \
                 tc.tile_pool(name="prms", bufs=1, space="PSUM") as prms:
                x_sb = xp.tile([128, DT * T], BF16, tag="x")
                xv = x_sb[:].rearrange("p (dt t) -> p dt t", t=T)
                xs_d = xt_d.ap().rearrange("(dt p) t -> p dt t", p=128)
                for et in range(ET):
                    ws = []
                    for nm, wd in (("wf", wf_d), ("wi", wi_d), ("wg", wg_d)):
                        w = wp.tile([128, DT * 128], BF16, tag=nm)
                        if et == 0 and nm == "wf":
                            # fine-grained first loads across parallel queues
                            nc.sync.dma_start(
                                w[:, 0:DT * 64],
                                wd.ap()[0:128, 0:DT * 64])
                            for q in range(4):
                                nc.sync.dma_start(
                                    xv[:, 4 * q:4 * (q + 1), 0:CH],
                                    xs_d[:, 4 * q:4 * (q + 1), 0:CH])
                            nc.sync.dma_start(
                                w[:, DT * 64:DT * 128],
                                wd.ap()[0:128, DT * 64:DT * 128])
                        else:
                            nc.sync.dma_start(
                                w[:], wd.ap()[et * 128:(et + 1) * 128, :])
                        ws.append(w)
                    wfv, wiv, wgv = ws
                    if et == 0:
                        for c in range(1, NC):
                            nc.sync.dma_start(xv[:, :, c * CH:(c + 1) * CH],
                                              xs_d[:, :, c * CH:(c + 1) * CH])
                        nc.sync.dma_start(gnw[:], gnw_d.ap()[:])
                        nc.sync.dma_start(maskt[:], mask_d.ap()[:])
                    if et == 1:
                        for dt in range(NWOP):
                            wo = pre2.tile([128, ET * 128], BF16, tag=f"wo{dt}")
                            nc.sync.dma_start(
                                wo[:], wo_d.ap()[dt * 128:(dt + 1) * 128, :])
                            wos[dt] = wo
                    o_et = hgp.tile([128, T], BF16, tag="osb")
                    for c in range(NC):
                        cs = slice(c * CH, (c + 1) * CH)
                        pf = pjf.tile([128, CH], F32, tag="pf")
                        pi = pji.tile([128, CH], F32, tag="pi")
                        pg = pji.tile([128, CH], F32, tag="pg")
                        for ps, w in ((pf, wfv), (pi, wiv), (pg, wgv)):
                            for dt in range(DT):
                                nc.tensor.matmul(
                                    ps[:], w[:, dt * 128:(dt + 1) * 128],
                                    x_sb[:, dt * T + c * CH:dt * T + (c + 1) * CH],
                                    start=(dt == 0), stop=(dt == DT - 1))
                        last_et = et == ET - 1
                        gate = wk.tile([128, CH], F32, tag="gate")
                        nc.scalar.activation(gate[:], pf[:], AF.Sigmoid)
                        sgi = wk.tile([128, CH], F32, tag="sgi")
                        omg = wk.tile([128, CH], F32, tag="omg")
                        sq = wk.tile([128, CH], F32, tag="sq")
                        iin = wk.tile([128, CH], F32, tag="iin")
                        if last_et:
                            # pi must be freed by the shallow Act queue: the
                            # phase-2 ypt banks reuse these pji banks
                            nc.scalar.activation(sgi[:], pi[:], AF.Silu)
                            nc.vector.tensor_scalar(omg[:], gate[:], -1.0,
                                                    1.0, OP.mult, OP.add)
                            nc.vector.tensor_mul(iin[:], sgi[:], omg[:])
                        else:
                            nc.scalar.activation(sgi[:], pi[:], AF.Sigmoid)
                            nc.vector.tensor_scalar(omg[:], gate[:], -1.0,
                                                    1.0, OP.mult, OP.add)
                            nc.vector.tensor_mul(sq[:], pi[:], sgi[:])
                            nc.vector.tensor_mul(iin[:], sq[:], omg[:])
                        htmp = wk.tile([128, CH], F32, tag="htmp")
                        nc.vector.tensor_tensor_scan(
                            htmp[:], gate[:], iin[:], carry[:, et:et + 1],
                            OP.mult, OP.add)
                        nc.vector.tensor_copy(carry[:, et:et + 1],
                                              htmp[:, CH - 1:CH])
                        if c == 0:
                            nc.vector.tensor_tensor_scan(
                                call[:, et * CLEN:(et + 1) * CLEN],
                                gate[:, 0:CLEN], gate[:, 0:CLEN], 1.0,
                                OP.mult, OP.bypass)
                            nc.vector.tensor_copy(
                                h0sb[:, et * CLEN:(et + 1) * CLEN],
                                htmp[:, 0:CLEN])
                        sgh = wk.tile([128, CH], F32, tag="sgh")
                        nc.scalar.activation(sgh[:], htmp[:], AF.Sigmoid)
                        swh = wk.tile([128, CH], F32, tag="swh")
                        nc.vector.tensor_mul(swh[:], htmp[:], sgh[:])
                        gsq = wk.tile([128, CH], F32, tag="gsq")
                        if last_et:
                            # pg likewise freed via Act, osb reads the copy
                            nc.scalar.copy(gsq[:], pg[:])
                            nc.vector.scalar_tensor_tensor(
                                o_et[:, cs], gsq[:], gnw[:, et:et + 1],
                                swh[:], OP.mult, OP.mult)
                        else:
                            nc.vector.scalar_tensor_tensor(
                                o_et[:, cs], pg[:], gnw[:, et:et + 1],
                                swh[:], OP.mult, OP.mult)
                            nc.vector.tensor_copy(gsq[:], pg[:])
                        if c == 0:
                            nc.vector.tensor_copy(
                                g0sb[:, et * CLEN:(et + 1) * CLEN],
                                gsq[:, 0:CLEN])
                        nc.vector.tensor_mul(sq[:], gsq[:], gsq[:])
                        nc.vector.tensor_add(acc[:, cs], acc[:, cs], sq[:])
                        if et == ET - 1:
                            accb = hgp.tile([128, CH], BF16, tag="accb")
                            nc.vector.tensor_copy(accb[:], acc[:, cs])
                            accbs[c] = accb
                            if c >= 1:   # deferred: never stalls the PE
                                emit_sum(c - 1, prms)
                    if et < ETS:
                        nc.sync.dma_start(osb_sp[et * 128:(et + 1) * 128, :],
                                          o_et[:])
                        p1 = pre2.tile([128, CH], BF16, tag=f"p1s{et}")
                        nc.vector.tensor_copy(p1[:], o_et[:, CH:2 * CH])
                        pre1[et] = p1
                    else:
                        osb_live[et] = o_et

            # ---------------- phase 1.5: carry exchange ---------------------
            nc.sync.dma_start(hl_i[:], carry[:])
            nc.gpsimd.collective_compute(
                "AllReduce", OP.add,
                replica_groups=[[0, 1], [2, 3], [4, 5], [6, 7]],
                ins=[hl_i.opt()], outs=[hl_o.opt()])

            # ---------------- phase 2: output projection --------------------
            # chunk 0 (needs the carry fixup) last, so the collective hides
            corder = (1, 2, 3, 0)
            ets_order = [ET - 2] + list(range(ETS)) + [ET - 1]
            strips = {}
            with tc.tile_pool(name="wop", bufs=1) as wop, \
                 tc.tile_pool(name="oscp", bufs=2) as oscp, \
                 tc.tile_pool(name="w2", bufs=3) as w2, \
                 tc.tile_pool(name="ycp", bufs=3) as ycp, \
                 tc.tile_pool(name="yp", bufs=4, space="PSUM") as yp:
                for dt in range(NWOP, DT):
                    wo = wop.tile([128, ET * 128], BF16, tag=f"wo{dt}")
                    nc.sync.dma_start(wo[:], wo_d.ap()[dt * 128:(dt + 1) * 128, :])
                    wos[dt] = wo

                def load_strips(c):
                    for et in range(ETS):
                        t = oscp.tile([128, CH], BF16, tag=f"os{et}")
                        nc.sync.dma_start(
                            t[:], osb_sp[et * 128:(et + 1) * 128,
                                         c * CH:(c + 1) * CH])
                        strips[(c, et)] = t

                load_strips(2)
                # rsqrt on the (idle) Act engine, first-needed chunk first
                for c in (1, 2, 0):
                    nc.scalar.activation(rms[:, c * CH:(c + 1) * CH],
                                         ms[c][:], AF.Abs_reciprocal_sqrt)
                for ci, c in enumerate(corder):
                    cs = slice(c * CH, (c + 1) * CH)
                    if c == 2:
                        load_strips(3)
                    elif c == 3:
                        load_strips(0)
                    if c == 0:
                        # collective result -> cin, then fix chunk-0 prefixes
                        nc.sync.dma_start(recv[:], hl_o[:])
                        nc.vector.tensor_sub(recv[:], recv[:], carry[:])
                        nc.vector.tensor_scalar(cin[:], recv[:],
                                                maskt[:, 0:1], None, OP.mult)
                        for et in range(ET):
                            es = slice(et * CLEN, (et + 1) * CLEN)
                            hf = w2.tile([128, CLEN], F32, tag="hf")
                            nc.vector.tensor_copy(hf[:], h0sb[:, es])
                            cf = w2.tile([128, CLEN], F32, tag="cf")
                            nc.vector.tensor_copy(cf[:], call[:, es])
                            nc.vector.scalar_tensor_tensor(
                                hf[:], cf[:], cin[:, et:et + 1], hf[:],
                                OP.mult, OP.add)
                            swf = w2.tile([128, CLEN], F32, tag="swf")
                            nc.scalar.activation(swf[:], hf[:], AF.Silu)
                            gf0 = w2.tile([128, CLEN], F32, tag="gf0")
                            nc.vector.tensor_copy(gf0[:], g0sb[:, es])
                            dst = (strips[(0, et)][:, 0:CLEN] if et < ETS
                                   else osb_live[et][:, 0:CLEN])
                            nc.vector.scalar_tensor_tensor(
                                dst, gf0[:], gnw[:, et:et + 1], swf[:],
                                OP.mult, OP.mult)
                    for dt in range(DT):
                        ypt = yp.tile([128, CH], F32, tag="ypt")
                        for k, et in enumerate(ets_order):
                            if et >= ETS:
                                src = osb_live[et][:, cs]
                            elif c == 1:
                                src = pre1[et][:]
                            else:
                                src = strips[(c, et)][:]
                            nc.tensor.matmul(
                                ypt[:], wos[dt][:, et * 128:(et + 1) * 128],
                                src, start=(k == 0), stop=(k == ET - 1))
                        ych = ycp.tile([128, CH], BF16, tag="ych")
                        nc.vector.tensor_mul(ych[:], ypt[:], rms[:, cs])
                        nc.sync.dma_start(
                            yt_d.ap()[dt * 128:(dt + 1) * 128, cs], ych[:])
                        if ci == 0 and dt == 2:
                            # last rms chunk, off the hot path
                            S3 = yp.tile([128, CH], F32, tag="S")
                            nc.tensor.matmul(S3[:], onesb[:], accbs[3][:],
                                             start=True, stop=True)
                            nc.scalar.activation(ms[3][:], S3[:], AF.Copy,
                                                 bias=EPS, scale=1.0 / D)
                            nc.scalar.activation(rms[:, 3 * CH:4 * CH],
                                                 ms[3][:],
                                                 AF.Abs_reciprocal_sqrt)
    nc.compile()
    return nc


def _get_nc():
    if "nc" not in _CACHE:
        _CACHE["nc"] = _build()
    return _CACHE["nc"]


def _tile_lhs(WT):
    """[K*128, M*128] host mat -> pre-tiled [M_blk*128, K_blk*128] so row block
    m is the lhsT tile [128, K*128] with columns grouped by contraction block."""
    KB = WT.shape[0] // 128
    MB = WT.shape[1] // 128
    t = WT.reshape(KB, 128, MB, 128).transpose(2, 1, 0, 3)
    return np.ascontiguousarray(t.reshape(MB * 128, KB * 128)
                                .astype(ml_dtypes.bfloat16))


def kernel(hidden_states, Wi, Wf, Wg, g_norm_weight, Wo, **_unused):
    nc = _get_nc()
    bf = ml_dtypes.bfloat16
    wiT = _tile_lhs(np.ascontiguousarray(Wi.T))
    wfT = _tile_lhs(np.ascontiguousarray(Wf.T))
    wgT = _tile_lhs(np.ascontiguousarray(Wg.T))
    woT = _tile_lhs(np.ascontiguousarray(Wo.T))
    gnw = np.ascontiguousarray(
        np.asarray(g_norm_weight, np.float32).reshape(ET, 128).T)
    in_maps = []
    for c in range(NCORE):
        b, half = c // 2, c % 2
        xt = np.ascontiguousarray(
            hidden_states[b, half * T:(half + 1) * T, :].T.astype(bf))
        mask = np.full((128, 1), float(half), np.float32)
        in_maps.append({"xt": xt, "wi": wiT, "wf": wfT, "wg": wgT,
                        "wo": woT, "gnw": gnw, "mask": mask})
    _CACHE["in_maps"] = in_maps
    res = run_bass_kernel_spmd(nc, in_maps, list(range(NCORE))).results
    y = np.empty((B, L, D), np.float32)
    for c in range(NCORE):
        b, half = c // 2, c % 2
        y[b, half * T:(half + 1) * T, :] = res[c]["yt"].astype(np.float32).T
    return y
